# revision 1
# baseline (speedup 1.0000x reference)
"""Trainium2 Bass kernel for nn_DepthMemoryCache.

Reference computation (D=8, B=4, S=4096, C=1024, G=64):
    u     = einsum('bsc,gc->bsg', x[-1], W_u)
    keys  = einsum('dbc,gc->dbg', x.mean(2), W_u)
    gates = softmax(einsum('bsg,dbg->bsd', u, keys), axis=-1)
    out   = einsum('dbsc,bsd->bsc', x, gates)

Strategy: shard the sequence axis over 8 cores (core i gets
x[:, :, i*512:(i+1)*512, :]). Per core, two streaming passes over the 64MB
shard:
  A) depth/batch sums over s on PE: slabs are cast to bf16 (on the otherwise
     idle DVE/ACT engines) and column-summed with indicator stationaries in a
     single PSUM accumulation region. For the d=D-1 slabs, uT = W_u @ x7.T is
     also computed on PE (bf16 transposes + matmuls) so phase B needs no
     per-block transposes. A 128KB all-core AllReduce completes the
     full-sequence means (a tiny warm-up AllReduce at kernel start absorbs
     comm setup under phase A; collective bounce DMAs ride GpSimd's queue so
     the Sync engine keeps issuing prefetch reads).
  B) after a short fixup (meanT transposes + keysT matmuls), each 128-row
     block needs ONE small matmul for logits, softmax via ACT exp with
     accum_out, then 8 streamed depth tiles combined by fused
     scalar_tensor_tensor FMAs (fp32, exact) with per-partition gate scalars
     on DVE; gates are interleaved with streaming so the first FMA fires
     right after the collective.
HBM traffic per core: 64 (A) + 64 (B) + 8 (write) = 136MB.
The bf16 mean/logit paths cost ~1e-3/2e-4 relative on gates only; the output
weighted sum stays fp32.
"""
import sys

sys.path.insert(0, "/opt/trn_rl_repo")

from contextlib import ExitStack

import numpy as np
from concourse import bacc, bass, mybir, tile, masks
from concourse import bass_utils

F32 = mybir.dt.float32
BF16 = mybir.dt.bfloat16

D, B, S, C, G = 8, 4, 4096, 1024, 64
N_CORES = 8
P = 128                 # partition count / block rows
NKC = C // P            # 8 column chunks of 128


def build_body(tc, x, w, y, s_sh):
    """Emit the kernel IR. x:[D,B,s_sh,C], w:[G,C], y:[B,s_sh,C] dram APs."""
    nc = tc.nc
    nj = s_sh // P      # 128-row blocks per (d, b)
    mul, add = mybir.AluOpType.mult, mybir.AluOpType.add
    DB = D * B
    es = ExitStack()

    singles = es.enter_context(tc.tile_pool(name="singles", bufs=1))
    ident = singles.tile([P, P], F32)
    masks.make_identity(nc, ident[:])
    ident_bf = singles.tile([P, P], BF16)
    masks.make_identity(nc, ident_bf[:])
    # indicator stationaries: ind[:, r, m] = (m == r) / S  — column-sums a
    # bf16 slab into psum row r with one N=512 matmul per c-half.
    ind_bf = singles.tile([P, DB, DB], BF16)
    nc.vector.memset(ind_bf[:], 0.0)
    for r in range(DB):
        nc.vector.memset(ind_bf[:, r, r:r + 1], 1.0 / (N_CORES * s_sh))
    w_sb = singles.tile([G, C], F32)
    nc.sync.dma_start(w_sb[:], w[:])
    x7bf_sb = singles.tile([P, B, nj, C], BF16)
    gates_sb = singles.tile([P, B, nj, D], F32)
    sums_sb = singles.tile([DB, C], F32)
    sumk_sb = singles.tile([G, B * D], F32)
    meanT_sb = singles.tile([P, NKC * DB], F32)
    wT_sb = singles.tile([P, NKC, G], F32)
    wT_bf = singles.tile([P, NKC, G], BF16)
    keysT_sb = singles.tile([G, B, D], F32)
    uT_sb = singles.tile([G, B, nj, P], F32)

    stream = es.enter_context(tc.tile_pool(name="stream", bufs=3))
    bfp = es.enter_context(tc.tile_pool(name="bfp", bufs=2))

    dram = es.enter_context(tc.tile_pool(name="dram", bufs=1, space="DRAM"))
    # tiny warm-up AllReduce: absorbs collective-comm setup under phase A
    ccw_in = dram.tile([1, 16], F32)
    ccw_out = dram.tile([1, 16], F32)
    cc_in = dram.tile([G, B * D], F32)
    cc_out = dram.tile([G, B * D], F32)
    warm_sb = singles.tile([1, 16], F32)
    nc.vector.memset(warm_sb[:], 0.0)
    nc.gpsimd.dma_start(ccw_in[:], warm_sb[:])
    nc.gpsimd.collective_compute(
        "AllReduce", add, replica_groups=[list(range(N_CORES))],
        ins=[ccw_in.opt()], outs=[ccw_out.opt()],
    )

    # ---------------- Phase A: partial sums over s (scaled by 1/S) ----------
    with tc.tile_pool(name="psumA", bufs=1, space="PSUM") as psA, \
         tc.tile_pool(name="psumT", bufs=1, space="PSUM") as psT, \
         tc.tile_pool(name="psumXA", bufs=3, space="PSUM") as psXA, \
         tc.tile_pool(name="psumU", bufs=2, space="PSUM") as psU, \
         tc.tile_pool(name="xtA", bufs=3) as xtA:
        sums_ps = psA.tile([DB, C], F32)

        # Each 512-col half of sums_ps is one 2KB PSUM zero region: start=True
        # zeroes the WHOLE region, so exactly one start (global first MM into
        # that region) / one stop (global last); every other matmul
        # accumulates onto pending-zero bytes. Rows m != r get +0.
        def sum_slab(slab_bf, d, b, first, last):
            r = d * B + b
            for h in range(2):
                for j in range(nj):
                    nc.tensor.matmul(
                        sums_ps[:, h * 512:(h + 1) * 512],
                        ind_bf[:, r, :],
                        slab_bf[:, j, h * 512:(h + 1) * 512],
                        start=(first and j == 0),
                        stop=(last and j == nj - 1),
                    )

        def cast_slab(dst_bf, src_f32, i):
            # split the fp32->bf16 casts between DVE and ACT (both idle here)
            for j in range(nj):
                if (i * nj + j) % 2 == 0:
                    nc.vector.tensor_copy(dst_bf[:, j, :], src_f32[:, j, :])
                else:
                    nc.scalar.copy(dst_bf[:, j, :], src_f32[:, j, :])

        # one-time W_u transpose: wT[c, g] chunks (fp32 + bf16 copies)
        for k in range(NKC):
            tr = psT.tile([P, NKC * DB], F32, tag="fix")
            nc.tensor.transpose(tr[:, :G], w_sb[:, k * P:(k + 1) * P], ident[:G, :G])
            nc.vector.tensor_copy(wT_sb[:, k, :], tr[:, :G])
            nc.scalar.copy(wT_bf[:, k, :], tr[:, :G])

        def u_block(b, j):
            # uT[g, s-block] = sum_k (wT_k).T @ x7T_k on PE (reads resident
            # x7bf, so this can run any time after the d=D-1 cast)
            u_ps = psU.tile([G, P], F32, tag="u")
            for k in range(NKC):
                xt_ps = psXA.tile([P, P], BF16, tag="xt_ps")
                nc.tensor.transpose(
                    xt_ps[:], x7bf_sb[:, b, j, k * P:(k + 1) * P],
                    ident_bf[:])
                xt_sb = xtA.tile([P, P], BF16, tag="xt_sb")
                if k % 2 == 0:
                    nc.scalar.copy(xt_sb[:], xt_ps[:])
                else:
                    nc.vector.tensor_copy(xt_sb[:], xt_ps[:])
                nc.tensor.matmul(
                    u_ps[:], wT_bf[:, k, :], xt_sb[:],
                    start=(k == 0), stop=(k == NKC - 1))
            nc.vector.tensor_copy(uT_sb[:, b, j, :], u_ps[:])

        # d = 7 first (fills the resident x7bf); one uT block is interleaved
        # after every later slab so the PE/copy work spreads over phase A
        ublocks = [(b, j) for b in range(B) for j in range(nj)]
        ub_i = 0
        for dd in range(D):
            d = (dd + D - 1) % D
            for b in range(B):
                slab = stream.tile([P, nj, C], F32, tag="slab")
                nc.sync.dma_start(
                    slab[:], x[d, b].rearrange("(j p) c -> p j c", p=P))
                if d == D - 1:
                    xbf = x7bf_sb[:, b]
                else:
                    xbf_t = bfp.tile([P, nj, C], BF16, tag="xbf")
                    xbf = xbf_t[:]
                cast_slab(xbf, slab[:], d * B + b)
                sum_slab(xbf, d, b, first=(dd == 0 and b == 0),
                         last=(dd == D - 1 and b == B - 1))
                if dd >= 1 and ub_i < len(ublocks):
                    ub, uj = ublocks[ub_i]; ub_i += 1
                    u_block(ub, uj)
        while ub_i < len(ublocks):
            ub, uj = ublocks[ub_i]; ub_i += 1
            u_block(ub, uj)

        nc.vector.tensor_copy(sums_sb[:], sums_ps[:])

        # ---- local partial keysT (keys are linear in the means, so the ----
        # ---- AllReduce can run in the tiny keys space: 8KB not 128KB)  ----
        # meanT[c, (d,b)] chunks via PE transpose — all 8 into one psum tile
        # (one zero region => single start/stop accumulation group)
        mt_ps = psT.tile([P, NKC * DB], F32, tag="fix")
        for k in range(NKC):
            nc.tensor.matmul(
                mt_ps[:, k * DB:(k + 1) * DB],
                sums_sb[:, k * P:(k + 1) * P], ident[:DB, :DB],
                is_transpose=True, start=(k == 0), stop=(k == NKC - 1))
        nc.vector.tensor_copy(meanT_sb[:], mt_ps[:])
        # partial keysT[g, d] per b = sum_k wT_k.T @ meanT_k
        keys_ps = psT.tile([P, NKC * DB], F32, tag="fix")
        for b in range(B):
            for k in range(NKC):
                nc.tensor.matmul(
                    keys_ps[:G, b * D:(b + 1) * D],
                    wT_sb[:, k, :],
                    meanT_sb[:, k * DB:(k + 1) * DB].rearrange(
                        "p (d b) -> p d b", b=B)[:, :, b],
                    start=(k == 0), stop=(k == NKC - 1),
                )
        nc.vector.tensor_copy(sumk_sb[:], keys_ps[:G, :B * D])

    # ---------------- AllReduce the [G, B*D] partial keys -------------------
    # bounce DMAs go through GpSimd's queue so the Sync engine never blocks
    # on the collective and keeps issuing phase-B prefetch reads.
    nc.gpsimd.dma_start(cc_in[:], sumk_sb[:])
    nc.gpsimd.collective_compute(
        "AllReduce", add,
        replica_groups=[list(range(N_CORES))],
        ins=[cc_in.opt()], outs=[cc_out.opt()],
    )
    nc.gpsimd.dma_start(
        keysT_sb[:].rearrange("g b d -> g (b d)"), cc_out[:])

    # ---------------- Phase B: gates + depth-weighted sum -------------------
    with tc.tile_pool(name="psumL", bufs=2, space="PSUM") as psL, \
         tc.tile_pool(name="bstream", bufs=14) as bstream, \
         tc.tile_pool(name="accp", bufs=4) as accp, \
         tc.tile_pool(name="small", bufs=4) as small:
        for b in range(B):
            for j in range(nj):
                # logits for this block: one small matmul off resident uT
                lg_ps = psL.tile([P, D], F32, tag="lg")
                nc.tensor.matmul(lg_ps[:], uT_sb[:, b, j, :], keysT_sb[:, b, :])
                e_sb = small.tile([P, D], F32, tag="e")
                z_sb = small.tile([P, 1], F32, tag="z")
                rz_sb = small.tile([P, 1], F32, tag="rz")
                nc.scalar.activation(
                    e_sb[:], lg_ps[:], mybir.ActivationFunctionType.Exp,
                    accum_out=z_sb[:])
                nc.vector.reciprocal(rz_sb[:], z_sb[:])
                nc.scalar.mul(gates_sb[:, b, j, :], e_sb[:], rz_sb[:])

                acc = accp.tile([P, C], F32, tag="acc")
                for dd in range(D):
                    d = (dd + D - 1) % D        # d = 7 first, then 0..6
                    t = bstream.tile([P, C], F32, tag="bslab")
                    nc.sync.dma_start(
                        t[:], x[d, b, j * P:(j + 1) * P, :])
                    if dd == 0:
                        nc.vector.tensor_scalar_mul(
                            acc[:], t[:], gates_sb[:, b, j, d:d + 1])
                    else:
                        nc.vector.scalar_tensor_tensor(
                            out=acc[:], in0=t[:],
                            scalar=gates_sb[:, b, j, d:d + 1],
                            in1=acc[:], op0=mul, op1=add)
                # y writes via GpSimd (SWDGE): keeps both Sync's and ACT's
                # in-order queues free for prefetch reads / gate math
                nc.gpsimd.dma_start(y[b, j * P:(j + 1) * P, :], acc[:])

    es.close()


def build_nc(s_sh):
    nc = bacc.Bacc("TRN2", target_bir_lowering=False, debug=False,
                   num_devices=N_CORES)
    x_ap = nc.dram_tensor("x", [D, B, s_sh, C], F32, kind="ExternalInput").ap()
    w_ap = nc.dram_tensor("w", [G, C], F32, kind="ExternalInput").ap()
    y_ap = nc.dram_tensor("y", [B, s_sh, C], F32, kind="ExternalOutput").ap()
    with tile.TileContext(nc) as tc:
        build_body(tc, x_ap, w_ap, y_ap, s_sh)
    nc.compile()
    return nc


_NC_CACHE = {}


def _get_nc(s_sh):
    if s_sh not in _NC_CACHE:
        _NC_CACHE[s_sh] = build_nc(s_sh)
    return _NC_CACHE[s_sh]


def run(cached_states, W_u, trace=False, trace_cores=None):
    s_sh = S // N_CORES
    nc = _get_nc(s_sh)
    xs = np.asarray(cached_states, dtype=np.float32)
    ws = np.ascontiguousarray(np.asarray(W_u, dtype=np.float32))
    in_maps = []
    for i in range(N_CORES):
        sh = np.ascontiguousarray(xs[:, :, i * s_sh:(i + 1) * s_sh, :])
        in_maps.append({"x": sh, "w": ws})
    res = bass_utils.run_bass_kernel_spmd(
        nc, in_maps, core_ids=list(range(N_CORES)), trace=trace,
        trace_cores=trace_cores)
    out = np.empty((B, S, C), np.float32)
    for i in range(N_CORES):
        out[:, i * s_sh:(i + 1) * s_sh, :] = res.results[i]["y"]
    return out, res


def kernel(cached_states, W_u):
    out, _ = run(cached_states, W_u)
    return out



# revision 8
# speedup vs baseline: 1.2237x; 1.2237x over previous
"""Trainium2 Bass kernel for nn_DepthMemoryCache.

Reference computation (D=8, B=4, S=4096, C=1024, G=64):
    u     = einsum('bsc,gc->bsg', x[-1], W_u)
    keys  = einsum('dbc,gc->dbg', x.mean(2), W_u)
    gates = softmax(einsum('bsg,dbg->bsd', u, keys), axis=-1)
    out   = einsum('dbsc,bsd->bsc', x, gates)

Strategy: shard the sequence axis over 8 cores (core i gets
x[:, :, i*512:(i+1)*512, :]). Gates for batch b depend only on batch b's
means, so the kernel pipelines PER BATCH and reads HBM exactly once
(72MB/core total = 64 read + 8 write, vs 136MB for a two-pass scheme):

  For each b (staggered):
    A(b): stream the 8 depth slabs [512, C] once (16KB/partition
          descriptors via the (p j) row mapping), cast to a resident bf16
          SBUF cache (casts split ACT/DVE), column-sum via indicator
          matmuls on PE, and build uT = W @ x7^T on PE for d=7.
    AR(b): AllReduce the [G, D] partial keys for b (2KB) — fired from
          GpSimd right after b's sums; latency hides under A(b+1) stream.
    B(b): logits via one small PE matmul per 128-row block, softmax on
          ACT, then 32 scalar_tensor_tensor FMAs on DVE combining the 8
          cached bf16 depth slabs into fp32 output; y written via GpSimd.

SBUF cache = 2 batch generations x 8 slabs x [128, 4, 1024] bf16 = 16MB;
slab tiles recycle per-slab (bufs=16) so A(b+2) casts only wait for the
matching slab of B(b) to be combined. The bf16 cache costs ~1e-3 relative
error on the output; gates bf16 paths ~1e-3 on gates only.
"""
import sys

sys.path.insert(0, "/opt/trn_rl_repo")

from contextlib import ExitStack

import numpy as np
from concourse import bacc, bass, mybir, tile, masks
from concourse import bass_utils

F32 = mybir.dt.float32
BF16 = mybir.dt.bfloat16

D, B, S, C, G = 8, 4, 4096, 1024, 64
N_CORES = 8
P = 128                 # partition count / block rows
NKC = C // P            # 8 column chunks of 128


def build_body(tc, x, w, y, s_sh):
    """Emit the kernel IR. x:[D,B,s_sh,C], w:[G,C], y:[B,s_sh,C] dram APs."""
    nc = tc.nc
    nj = s_sh // P      # 4 row-chunks per partition
    mul, add = mybir.AluOpType.mult, mybir.AluOpType.add
    es = ExitStack()

    singles = es.enter_context(tc.tile_pool(name="singles", bufs=1))
    ident = singles.tile([P, P], F32)
    masks.make_identity(nc, ident[:])
    ident_bf = singles.tile([P, P], BF16)
    masks.make_identity(nc, ident_bf[:])
    # indicator stationaries: ind[:, r, m] = (m == r) / S — column-sums a
    # bf16 slab into psum row r (r = depth index here).
    ind_bf = singles.tile([P, D, D], BF16)
    nc.vector.memset(ind_bf[:], 0.0)
    for r in range(D):
        nc.vector.memset(ind_bf[:, r, r:r + 1], 1.0 / (N_CORES * s_sh))
    w_sb = singles.tile([G, C], F32)
    nc.sync.dma_start(w_sb[:], w[:])
    gates_sb = singles.tile([P, B, nj, D], F32)
    wT_sb = singles.tile([P, NKC, G], F32)
    wT_bf = singles.tile([P, NKC, G], BF16)
    keysT_sb = singles.tile([G, B, D], F32)
    keysT_bf = singles.tile([G, B, D], BF16)
    sumk_sb = singles.tile([G, B, D], F32)
    uT_sb = singles.tile([G, B, nj, P], BF16)

    # bf16 slab cache: 2 batch generations in flight (16 slabs x 8KB/part)
    cache = es.enter_context(tc.tile_pool(name="cache", bufs=2 * D))
    stage = es.enter_context(tc.tile_pool(name="stage", bufs=2))
    accp = es.enter_context(tc.tile_pool(name="accp", bufs=2))
    sumsp = es.enter_context(tc.tile_pool(name="sumsp", bufs=2))

    dram = es.enter_context(tc.tile_pool(name="dram", bufs=1, space="DRAM"))
    # tiny warm-up AllReduce: absorbs collective-comm setup under A(b0)
    ccw_in = dram.tile([1, 16], F32)
    ccw_out = dram.tile([1, 16], F32)
    cc_in, cc_out = [], []
    for b in range(B):
        cc_in_b = dram.tile([G, D], F32, tag=f"ci{b}", name=f"cc_in_{b}")
        cc_out_b = dram.tile([G, D], F32, tag=f"co{b}", name=f"cc_out_{b}")
        cc_in.append(cc_in_b)
        cc_out.append(cc_out_b)
    warm_sb = singles.tile([1, 16], F32)
    nc.vector.memset(warm_sb[:], 0.0)
    nc.gpsimd.dma_start(ccw_in[:], warm_sb[:])
    nc.gpsimd.collective_compute(
        "AllReduce", add, replica_groups=[list(range(N_CORES))],
        ins=[ccw_in.opt()], outs=[ccw_out.opt()],
    )

    psS = es.enter_context(tc.tile_pool(name="psumS", bufs=2, space="PSUM"))
    psT = es.enter_context(tc.tile_pool(name="psumT", bufs=1, space="PSUM"))
    psU = es.enter_context(tc.tile_pool(name="psumU", bufs=1, space="PSUM"))
    psF = es.enter_context(tc.tile_pool(name="psumF", bufs=1, space="PSUM"))
    psL = es.enter_context(tc.tile_pool(name="psumL", bufs=1, space="PSUM"))
    xtp = es.enter_context(tc.tile_pool(name="xtp", bufs=2))

    # one-time W_u transpose: wT[c, g] chunks (fp32 + bf16 copies)
    for k in range(NKC):
        tr = psF.tile([P, G], F32, tag="fix")
        nc.tensor.transpose(tr[:, :G], w_sb[:, k * P:(k + 1) * P], ident[:G, :G])
        nc.vector.tensor_copy(wT_sb[:, k, :], tr[:, :G])
        nc.scalar.copy(wT_bf[:, k, :], tr[:, :G])

    cache_tiles = {}            # (b, d) -> bf16 slab tile [P, nj, C]

    def emit_A(b):
        """Stream batch b's 8 depth slabs; sums into psum; uT for d=7."""
        sums_ps = psS.tile([D, C], F32, tag="sums")
        ublocks = list(range(nj))
        dorder = [D - 1] + list(range(D - 1))
        for di, d in enumerate(dorder):
            slab = stage.tile([P, nj, C], F32, tag="slab")
            nc.sync.dma_start(
                slab[:], x[d, b].rearrange("(p j) c -> p j c", j=nj))
            xbf = cache.tile([P, nj, C], BF16, tag="cslab")
            cache_tiles[(b, d)] = xbf
            # casts all on ACT: DVE is reserved for phase-B combines, so
            # cast pacing never queues behind a combine burst
            for j in range(nj):
                nc.scalar.copy(xbf[:, j, :], slab[:, j, :])
            # column sums over this slab's 512 rows into psum row d
            for h in range(2):
                for j in range(nj):
                    nc.tensor.matmul(
                        sums_ps[:, h * 512:(h + 1) * 512],
                        ind_bf[:, d, :],
                        xbf[:, j, h * 512:(h + 1) * 512],
                        start=(di == 0 and j == 0),
                        stop=(di == D - 1 and j == nj - 1),
                    )
            # one uT block after each of slabs d=0,1,2,3 (x7bf is resident)
            if 1 <= di <= nj:
                j = ublocks[di - 1]
                x7bf = cache_tiles[(b, D - 1)]
                u_ps = psU.tile([G, P], F32, tag="u")
                for k in range(NKC):
                    xt_ps = psT.tile([P, P], BF16, tag="xt")
                    nc.tensor.transpose(
                        xt_ps[:], x7bf[:, j, k * P:(k + 1) * P], ident_bf[:])
                    xt_sb = xtp.tile([P, P], BF16, tag="xt_sb")
                    nc.vector.tensor_copy(xt_sb[:], xt_ps[:])
                    nc.tensor.matmul(
                        u_ps[:], wT_bf[:, k, :], xt_sb[:],
                        start=(k == 0), stop=(k == NKC - 1))
                nc.scalar.copy(uT_sb[:, b, j, :], u_ps[:])

        # fixup: sums -> meanT chunks -> partial keysT; AR via gpsimd
        sums_sb = sumsp.tile([D, C], F32, tag="sums_sb")
        nc.vector.tensor_copy(sums_sb[:], sums_ps[:])
        mt_ps = psF.tile([P, NKC * D], F32, tag="fix")
        for k in range(NKC):
            nc.tensor.matmul(
                mt_ps[:, k * D:(k + 1) * D],
                sums_sb[:, k * P:(k + 1) * P], ident[:D, :D],
                is_transpose=True, start=(k == 0), stop=(k == NKC - 1))
        meanT_tmp = xtp.tile([P, NKC * D], F32, tag="mt")
        nc.vector.tensor_copy(meanT_tmp[:], mt_ps[:])
        keys_ps = psF.tile([P, NKC * D], F32, tag="fix")
        for k in range(NKC):
            nc.tensor.matmul(
                keys_ps[:G, :D],
                wT_sb[:, k, :],
                meanT_tmp[:, k * D:(k + 1) * D],
                start=(k == 0), stop=(k == NKC - 1))
        nc.vector.tensor_copy(sumk_sb[:, b, :], keys_ps[:G, :D])
        nc.gpsimd.dma_start(cc_in[b][:], sumk_sb[:, b, :])
        nc.gpsimd.collective_compute(
            "AllReduce", add, replica_groups=[list(range(N_CORES))],
            ins=[cc_in[b].opt()], outs=[cc_out[b].opt()],
        )

    def emit_B(b):
        """Gates + depth-weighted combine for batch b from the bf16 cache."""
        nc.gpsimd.dma_start(keysT_sb[:, b, :], cc_out[b][:])
        nc.vector.tensor_copy(keysT_bf[:, b, :], keysT_sb[:, b, :])
        small = xtp  # reuse pool for tiny tiles
        for j in range(nj):
            lg_ps = psL.tile([P, D], F32, tag="lg")
            nc.tensor.matmul(lg_ps[:], uT_sb[:, b, j, :], keysT_bf[:, b, :])
            e_sb = small.tile([P, D], F32, tag="e")
            z_sb = small.tile([P, 1], F32, tag="z")
            rz_sb = small.tile([P, 1], F32, tag="rz")
            nc.scalar.activation(
                e_sb[:], lg_ps[:], mybir.ActivationFunctionType.Exp,
                accum_out=z_sb[:])
            nc.vector.reciprocal(rz_sb[:], z_sb[:])
            nc.scalar.mul(gates_sb[:, b, j, :], e_sb[:], rz_sb[:])
        # combine halves: acc[P, 2, C] so y-writes are 8KB/partition
        for h in range(2):
            acc = accp.tile([P, 2, C], F32, tag="acc")
            for di, d in enumerate([D - 1] + list(range(D - 1))):
                xbf = cache_tiles.pop((b, d)) if h == 1 else cache_tiles[(b, d)]
                for jj in range(2):
                    j = h * 2 + jj
                    if di == 0:
                        nc.vector.tensor_scalar_mul(
                            acc[:, jj, :], xbf[:, j, :],
                            gates_sb[:, b, j, d:d + 1])
                    else:
                        nc.vector.scalar_tensor_tensor(
                            out=acc[:, jj, :], in0=xbf[:, j, :],
                            scalar=gates_sb[:, b, j, d:d + 1],
                            in1=acc[:, jj, :], op0=mul, op1=add)
            nc.gpsimd.dma_start(
                y[b].rearrange("(p j) c -> p j c", j=nj)[:, 2 * h:2 * h + 2, :],
                acc[:])

    emit_A(0)
    for b in range(1, B):
        emit_A(b)
        emit_B(b - 1)
    emit_B(B - 1)

    es.close()


def build_nc(s_sh):
    nc = bacc.Bacc("TRN2", target_bir_lowering=False, debug=False,
                   num_devices=N_CORES)
    x_ap = nc.dram_tensor("x", [D, B, s_sh, C], F32, kind="ExternalInput").ap()
    w_ap = nc.dram_tensor("w", [G, C], F32, kind="ExternalInput").ap()
    y_ap = nc.dram_tensor("y", [B, s_sh, C], F32, kind="ExternalOutput").ap()
    with tile.TileContext(nc) as tc:
        build_body(tc, x_ap, w_ap, y_ap, s_sh)
    nc.compile()
    return nc


_NC_CACHE = {}


def _get_nc(s_sh):
    if s_sh not in _NC_CACHE:
        _NC_CACHE[s_sh] = build_nc(s_sh)
    return _NC_CACHE[s_sh]


def run(cached_states, W_u, trace=False, trace_cores=None):
    s_sh = S // N_CORES
    nc = _get_nc(s_sh)
    xs = np.asarray(cached_states, dtype=np.float32)
    ws = np.ascontiguousarray(np.asarray(W_u, dtype=np.float32))
    in_maps = []
    for i in range(N_CORES):
        sh = np.ascontiguousarray(xs[:, :, i * s_sh:(i + 1) * s_sh, :])
        in_maps.append({"x": sh, "w": ws})
    res = bass_utils.run_bass_kernel_spmd(
        nc, in_maps, core_ids=list(range(N_CORES)), trace=trace,
        trace_cores=trace_cores)
    out = np.empty((B, S, C), np.float32)
    for i in range(N_CORES):
        out[:, i * s_sh:(i + 1) * s_sh, :] = res.results[i]["y"]
    return out, res


def kernel(cached_states, W_u):
    out, _ = run(cached_states, W_u)
    return out


# revision 10
# speedup vs baseline: 1.3372x; 1.0927x over previous
"""Trainium2 Bass kernel for nn_DepthMemoryCache.

Reference computation (D=8, B=4, S=4096, C=1024, G=64):
    u     = einsum('bsc,gc->bsg', x[-1], W_u)
    keys  = einsum('dbc,gc->dbg', x.mean(2), W_u)
    gates = softmax(einsum('bsg,dbg->bsd', u, keys), axis=-1)
    out   = einsum('dbsc,bsd->bsc', x, gates)

Strategy: shard the sequence axis over 8 cores (core i gets
x[:, :, i*512:(i+1)*512, :]). Gates for batch b depend only on batch b's
means, so the kernel pipelines PER BATCH and reads HBM exactly once
(72MB/core total = 64 read + 8 write, vs 136MB for a two-pass scheme):

  A(b): stream the 8 depth slabs [512, C] once (16KB/partition
        descriptors via the (p j) row mapping), cast to a resident bf16
        SBUF cache on ACT, j-reduce each slab on DVE (bf16 2x fast mode)
        so the PE indicator-matmul sums see 4x fewer moving columns, and
        build uT = W @ x7^T on PE for d=7.
  AR(b): AllReduce the [G, D] partial keys for b (2KB) from GpSimd right
        after b's sums; latency hides under A(b+1)'s stream.
  B(b): logits via one small PE matmul per 128-row block, softmax on ACT,
        then per-j chains of scalar_tensor_tensor FMAs on DVE that
        accumulate in PSUM (one SBUF + one PSUM source = full DVE rate;
        two SBUF sources would halve it), final FMA lands in SBUF ystage
        and GpSimd writes y with 8KB/partition descriptors.

B(b-1) emission is interleaved into A(b)'s slab loop (prelude after slab
5, combine chains after slabs 6/7 and post-fixup) so in-order engine
queues never stall A(b)'s critical path: ACT never waits on combines,
and the AR trigger is gated only by ACT fixup copies, not DVE backlog.
SBUF cache = 2 generations x 8 slabs x [128, 4, 1024] bf16 = 16MB.
bf16 cache/means cost ~2e-3 relative error on the output.
"""
import sys

sys.path.insert(0, "/opt/trn_rl_repo")

from contextlib import ExitStack

import numpy as np
from concourse import bacc, bass, mybir, tile, masks
from concourse import bass_utils

F32 = mybir.dt.float32
BF16 = mybir.dt.bfloat16

D, B, S, C, G = 8, 4, 4096, 1024, 64
N_CORES = 8
P = 128                 # partition count / block rows
NKC = C // P            # 8 column chunks of 128


def build_body(tc, x, w, y, s_sh):
    """Emit the kernel IR. x:[D,B,s_sh,C], w:[G,C], y:[B,s_sh,C] dram APs."""
    nc = tc.nc
    nj = s_sh // P      # 4 row-chunks per partition
    mul, add = mybir.AluOpType.mult, mybir.AluOpType.add
    es = ExitStack()

    singles = es.enter_context(tc.tile_pool(name="singles", bufs=1))
    ident = singles.tile([P, P], F32)
    masks.make_identity(nc, ident[:])
    ident_bf = singles.tile([P, P], BF16)
    masks.make_identity(nc, ident_bf[:])
    # indicator stationaries: ind[:, r, m] = (m == r) / S — column-sums a
    # bf16 j-reduced slab into psum row r (r = depth index).
    ind_bf = singles.tile([P, D, D], BF16)
    nc.vector.memset(ind_bf[:], 0.0)
    for r in range(D):
        nc.vector.memset(ind_bf[:, r, r:r + 1], 1.0 / (N_CORES * s_sh))
    w_sb = singles.tile([G, C], F32)
    nc.sync.dma_start(w_sb[:], w[:])
    gates_sb = singles.tile([P, B, nj, D], F32)
    wT_sb = singles.tile([P, NKC, G], F32)
    wT_bf = singles.tile([P, NKC, G], BF16)
    keysT_sb = singles.tile([G, B, D], F32)
    keysT_bf = singles.tile([G, B, D], BF16)
    sumk_sb = singles.tile([G, B, D], F32)
    uT_sb = singles.tile([G, B, nj, P], BF16)

    # bf16 slab cache: 2 batch generations in flight (16 slabs x 8KB/part)
    cache = es.enter_context(tc.tile_pool(name="cache", bufs=2 * D))
    stage = es.enter_context(tc.tile_pool(name="stage", bufs=2))
    jsump = es.enter_context(tc.tile_pool(name="jsump", bufs=2))
    ystp = es.enter_context(tc.tile_pool(name="ystp", bufs=2))

    dram = es.enter_context(tc.tile_pool(name="dram", bufs=1, space="DRAM"))
    # tiny warm-up AllReduce: absorbs collective-comm setup under A(b0)
    ccw_in = dram.tile([1, 16], F32)
    ccw_out = dram.tile([1, 16], F32)
    cc_in, cc_out = [], []
    for b in range(B):
        cc_in_b = dram.tile([G, D], F32, tag=f"ci{b}", name=f"cc_in_{b}")
        cc_out_b = dram.tile([G, D], F32, tag=f"co{b}", name=f"cc_out_{b}")
        cc_in.append(cc_in_b)
        cc_out.append(cc_out_b)
    warm_sb = singles.tile([1, 16], F32)
    nc.vector.memset(warm_sb[:], 0.0)
    nc.gpsimd.dma_start(ccw_in[:], warm_sb[:])
    nc.gpsimd.collective_compute(
        "AllReduce", add, replica_groups=[list(range(N_CORES))],
        ins=[ccw_in.opt()], outs=[ccw_out.opt()],
    )

    psS = es.enter_context(tc.tile_pool(name="psumS", bufs=1, space="PSUM"))
    psA = es.enter_context(tc.tile_pool(name="psumA", bufs=1, space="PSUM"))
    psT = es.enter_context(tc.tile_pool(name="psumT", bufs=1, space="PSUM"))
    psU = es.enter_context(tc.tile_pool(name="psumU", bufs=1, space="PSUM"))
    psF = es.enter_context(tc.tile_pool(name="psumF", bufs=1, space="PSUM"))
    psL = es.enter_context(tc.tile_pool(name="psumL", bufs=1, space="PSUM"))
    xtp = es.enter_context(tc.tile_pool(name="xtp", bufs=2))

    # one-time W_u transpose: wT[c, g] chunks (fp32 + bf16 copies)
    for k in range(NKC):
        tr = psF.tile([P, G], F32, tag="fix")
        nc.tensor.transpose(tr[:, :G], w_sb[:, k * P:(k + 1) * P], ident[:G, :G])
        nc.vector.tensor_copy(wT_sb[:, k, :], tr[:, :G])
        nc.scalar.copy(wT_bf[:, k, :], tr[:, :G])

    cache_tiles = {}            # (b, d) -> bf16 slab tile [P, nj, C]
    ystage = {}                 # (b, h) -> f32 tile [P, 2, C]

    def emit_slab(b, d, di, sums_ps):
        slab = stage.tile([P, nj, C], F32, tag="slab")
        nc.sync.dma_start(
            slab[:], x[d, b].rearrange("(p j) c -> p j c", j=nj))
        xbf = cache.tile([P, nj, C], BF16, tag="cslab")
        cache_tiles[(b, d)] = xbf
        # casts all on ACT: DVE is reserved for j-adds + combines
        for j in range(nj):
            nc.scalar.copy(xbf[:, j, :], slab[:, j, :])
        # j-reduction on DVE (bf16 2x fast mode): jsum = sum_j xbf[:, j, :]
        jsum = jsump.tile([P, C], BF16, tag="jsum")
        nc.vector.tensor_tensor(
            out=jsum[:], in0=xbf[:, 0, :], in1=xbf[:, 1, :], op=add)
        for j in range(2, nj):
            nc.vector.tensor_tensor(
                out=jsum[:], in0=jsum[:], in1=xbf[:, j, :], op=add)
        # column sums: 2 matmuls (512-col halves) into psum row d
        for h in range(2):
            nc.tensor.matmul(
                sums_ps[:, h * 512:(h + 1) * 512],
                ind_bf[:, d, :],
                jsum[:, h * 512:(h + 1) * 512],
                start=(di == 0),
                stop=(di == D - 1),
            )

    def emit_ublock(b, j):
        # uT[g, s-block] = sum_k (wT_k).T @ x7T_k on PE
        x7bf = cache_tiles[(b, D - 1)]
        u_ps = psU.tile([G, P], F32, tag="u")
        for k in range(NKC):
            xt_ps = psT.tile([P, P], BF16, tag="xt")
            nc.tensor.transpose(
                xt_ps[:], x7bf[:, j, k * P:(k + 1) * P], ident_bf[:])
            xt_sb = xtp.tile([P, P], BF16, tag="xt_sb")
            if k % 2 == 0:
                nc.scalar.copy(xt_sb[:], xt_ps[:])
            else:
                nc.vector.tensor_copy(xt_sb[:], xt_ps[:])
            nc.tensor.matmul(
                u_ps[:], wT_bf[:, k, :], xt_sb[:],
                start=(k == 0), stop=(k == NKC - 1))
        nc.scalar.copy(uT_sb[:, b, j, :], u_ps[:])

    def emit_fixup(b, sums_ps):
        # sums -> meanT chunks -> partial keysT; fixup copies on ACT so
        # the AR trigger is never gated by DVE combine backlog
        sums_sb = jsump.tile([D, C], F32, tag="sums_sb")
        nc.scalar.copy(sums_sb[:], sums_ps[:])
        mt_ps = psF.tile([P, NKC * D], F32, tag="fix")
        for k in range(NKC):
            nc.tensor.matmul(
                mt_ps[:, k * D:(k + 1) * D],
                sums_sb[:, k * P:(k + 1) * P], ident[:D, :D],
                is_transpose=True, start=(k == 0), stop=(k == NKC - 1))
        meanT_tmp = xtp.tile([P, NKC * D], F32, tag="mt")
        nc.scalar.copy(meanT_tmp[:], mt_ps[:])
        keys_ps = psF.tile([P, NKC * D], F32, tag="fix")
        for k in range(NKC):
            nc.tensor.matmul(
                keys_ps[:G, :D],
                wT_sb[:, k, :],
                meanT_tmp[:, k * D:(k + 1) * D],
                start=(k == 0), stop=(k == NKC - 1))
        nc.scalar.copy(sumk_sb[:, b, :], keys_ps[:G, :D])
        nc.gpsimd.dma_start(cc_in[b][:], sumk_sb[:, b, :])
        nc.gpsimd.collective_compute(
            "AllReduce", add, replica_groups=[list(range(N_CORES))],
            ins=[cc_in[b].opt()], outs=[cc_out[b].opt()],
        )

    def emit_prelude(b):
        # keysT fetch + logits + softmax for batch b
        nc.gpsimd.dma_start(keysT_sb[:, b, :], cc_out[b][:])
        nc.scalar.copy(keysT_bf[:, b, :], keysT_sb[:, b, :])
        for j in range(nj):
            lg_ps = psL.tile([P, D], F32, tag="lg")
            nc.tensor.matmul(lg_ps[:], uT_sb[:, b, j, :], keysT_bf[:, b, :])
            e_sb = xtp.tile([P, D], F32, tag="e")
            z_sb = xtp.tile([P, 1], F32, tag="z")
            rz_sb = xtp.tile([P, 1], F32, tag="rz")
            nc.scalar.activation(
                e_sb[:], lg_ps[:], mybir.ActivationFunctionType.Exp,
                accum_out=z_sb[:])
            nc.vector.reciprocal(rz_sb[:], z_sb[:])
            nc.scalar.mul(gates_sb[:, b, j, :], e_sb[:], rz_sb[:])

    def emit_chain(b, j):
        # weighted-depth combine for block j: STT chain with PSUM acc
        h, jj = j // 2, j % 2
        if jj == 0:
            yst_t = ystp.tile([P, 2, C], F32, tag="yst", name=f"yst_{b}_{h}")
            ystage[(b, h)] = yst_t
        yst = ystage[(b, h)]
        acc = psA.tile([P, C], F32, tag="acc")
        dorder = [D - 1] + list(range(D - 1))
        for di, d in enumerate(dorder):
            xbf = cache_tiles[(b, d)]
            g = gates_sb[:, b, j, d:d + 1]
            if di == 0:
                nc.vector.tensor_scalar_mul(acc[:], xbf[:, j, :], g)
            elif di < D - 1:
                nc.vector.scalar_tensor_tensor(
                    out=acc[:], in0=xbf[:, j, :], scalar=g, in1=acc[:],
                    op0=mul, op1=add)
            else:
                nc.vector.scalar_tensor_tensor(
                    out=yst[:, jj, :], in0=xbf[:, j, :], scalar=g, in1=acc[:],
                    op0=mul, op1=add)
        if j == 3:
            for d in range(D):
                del cache_tiles[(b, d)]

    def emit_ywrite(b, h):
        nc.gpsimd.dma_start(
            y[b].rearrange("(p j) c -> p j c", j=nj)[:, 2 * h:2 * h + 2, :],
            ystage.pop((b, h))[:])

    dorder = [D - 1] + list(range(D - 1))
    for b in range(B):
        sums_ps = psS.tile([D, C], F32, tag="sums")
        for si, d in enumerate(dorder):
            emit_slab(b, d, si, sums_ps)
            if 1 <= si <= nj:
                emit_ublock(b, si - 1)
            if b >= 1:
                if si == 5:
                    emit_prelude(b - 1)
                elif si == 6:
                    emit_chain(b - 1, 0)
                    emit_chain(b - 1, 1)
                    emit_ywrite(b - 1, 0)
                elif si == 7:
                    emit_chain(b - 1, 2)
        emit_fixup(b, sums_ps)
        if b >= 1:
            emit_chain(b - 1, 3)
            emit_ywrite(b - 1, 1)
    emit_prelude(B - 1)
    for j in range(nj):
        emit_chain(B - 1, j)
        if j % 2 == 1:
            emit_ywrite(B - 1, j // 2)

    es.close()


def build_nc(s_sh):
    nc = bacc.Bacc("TRN2", target_bir_lowering=False, debug=False,
                   num_devices=N_CORES)
    x_ap = nc.dram_tensor("x", [D, B, s_sh, C], F32, kind="ExternalInput").ap()
    w_ap = nc.dram_tensor("w", [G, C], F32, kind="ExternalInput").ap()
    y_ap = nc.dram_tensor("y", [B, s_sh, C], F32, kind="ExternalOutput").ap()
    with tile.TileContext(nc) as tc:
        build_body(tc, x_ap, w_ap, y_ap, s_sh)
    nc.compile()
    return nc


_NC_CACHE = {}


def _get_nc(s_sh):
    if s_sh not in _NC_CACHE:
        _NC_CACHE[s_sh] = build_nc(s_sh)
    return _NC_CACHE[s_sh]


def run(cached_states, W_u, trace=False, trace_cores=None):
    s_sh = S // N_CORES
    nc = _get_nc(s_sh)
    xs = np.asarray(cached_states, dtype=np.float32)
    ws = np.ascontiguousarray(np.asarray(W_u, dtype=np.float32))
    in_maps = []
    for i in range(N_CORES):
        sh = np.ascontiguousarray(xs[:, :, i * s_sh:(i + 1) * s_sh, :])
        in_maps.append({"x": sh, "w": ws})
    res = bass_utils.run_bass_kernel_spmd(
        nc, in_maps, core_ids=list(range(N_CORES)), trace=trace,
        trace_cores=trace_cores)
    out = np.empty((B, S, C), np.float32)
    for i in range(N_CORES):
        out[:, i * s_sh:(i + 1) * s_sh, :] = res.results[i]["y"]
    return out, res


def kernel(cached_states, W_u):
    out, _ = run(cached_states, W_u)
    return out


# revision 22
# speedup vs baseline: 1.4551x; 1.0882x over previous
"""Trainium2 Bass kernel for nn_DepthMemoryCache.

Reference computation (D=8, B=4, S=4096, C=1024, G=64):
    u     = einsum('bsc,gc->bsg', x[-1], W_u)
    keys  = einsum('dbc,gc->dbg', x.mean(2), W_u)
    gates = softmax(einsum('bsg,dbg->bsd', u, keys), axis=-1)
    out   = einsum('dbsc,bsd->bsc', x, gates)

Strategy: shard the sequence axis over 8 cores (core i gets
x[:, :, i*512:(i+1)*512, :]). Gates for batch b depend only on batch b's
means, so the kernel pipelines PER BATCH and reads HBM exactly once
(72MB/core total = 64 read + 8 write, vs 136MB for a two-pass scheme):

  A(b): stream the 8 depth slabs [512, C] once (16KB/partition
        descriptors via the (p j) row mapping), cast to a resident bf16
        SBUF cache on ACT, j-reduce each slab on DVE (bf16 2x fast mode)
        so the PE indicator-matmul sums see 4x fewer moving columns, and
        build uT = W @ x7^T on PE for d=7.
  AR(b): AllReduce the [G, D] partial keys for b (2KB) from GpSimd right
        after b's sums; latency hides under A(b+1)'s stream.
  B(b): logits via one small PE matmul per 128-row block, softmax on ACT,
        then per-j chains of scalar_tensor_tensor FMAs on DVE that
        accumulate in PSUM (one SBUF + one PSUM source = full DVE rate;
        two SBUF sources would halve it), final FMA lands in SBUF ystage
        and GpSimd writes y with 8KB/partition descriptors.

B(b-1) emission is interleaved into A(b)'s slab loop (prelude after slab
5, combine chains after slabs 6/7 and post-fixup) so in-order engine
queues never stall A(b)'s critical path: ACT never waits on combines,
and the AR trigger is gated only by ACT fixup copies, not DVE backlog.
SBUF cache = 2 generations x 8 slabs x [128, 4, 1024] bf16 = 16MB.
bf16 cache/means cost ~2e-3 relative error on the output.
"""
import sys

sys.path.insert(0, "/opt/trn_rl_repo")

from contextlib import ExitStack

import numpy as np
from concourse import bacc, bass, mybir, tile, masks
from concourse import bass_utils

F32 = mybir.dt.float32
BF16 = mybir.dt.bfloat16

D, B, S, C, G = 8, 4, 4096, 1024, 64
N_CORES = 8
P = 128                 # partition count / block rows
NKC = C // P            # 8 column chunks of 128
CV = 768                # combine column split: DVE gets [0:CV], GpSimd the rest


def build_body(tc, x, w, y, s_sh):
    """Emit the kernel IR. x:[D,B,s_sh,C], w:[G,C], y:[B,s_sh,C] dram APs."""
    nc = tc.nc
    nj = s_sh // P      # 4 row-chunks per partition
    mul, add = mybir.AluOpType.mult, mybir.AluOpType.add
    es = ExitStack()

    singles = es.enter_context(tc.tile_pool(name="singles", bufs=1))
    dram = es.enter_context(tc.tile_pool(name="dram", bufs=1, space="DRAM"))
    # warm-up AllReduce FIRST: the first cc trigger starts a ~43us barrier
    # and the cc stream serializes, so every microsecond earlier here pulls
    # AR(b0) earlier by the same amount
    ccw_in = dram.tile([1, 16], F32)
    ccw_out = dram.tile([1, 16], F32)
    warm_sb = singles.tile([1, 16], F32)
    nc.vector.memset(warm_sb[:], 0.0)
    nc.gpsimd.dma_start(ccw_in[:], warm_sb[:])
    nc.gpsimd.collective_compute(
        "AllReduce", add, replica_groups=[list(range(N_CORES))],
        ins=[ccw_in.opt()], outs=[ccw_out.opt()],
    )

    ident = singles.tile([P, P], F32)
    masks.make_identity(nc, ident[:])
    ident_bf = singles.tile([P, P], BF16)
    masks.make_identity(nc, ident_bf[:])
    # indicator stationaries: ind[:, r, m] = (m == r) / S — column-sums a
    # bf16 j-reduced slab into psum row r (r = depth index).
    ind_bf = singles.tile([P, D, D], BF16)
    nc.vector.memset(ind_bf[:], 0.0)
    for r in range(D):
        nc.vector.memset(ind_bf[:, r, r:r + 1], 1.0 / (N_CORES * s_sh))
    w_sb = singles.tile([G, C], F32)
    nc.sync.dma_start(w_sb[:], w[:])
    gates_sb = singles.tile([P, B, nj, D], F32)
    wT_sb = singles.tile([P, NKC, G], F32)
    wT_bf = singles.tile([P, NKC, G], BF16)
    keysT_sb = singles.tile([G, B, D], F32)
    keysT_bf = singles.tile([G, B, D], BF16)
    sumk_sb = singles.tile([G, B, D], F32)
    uT_sb = singles.tile([G, B, nj, P], BF16)

    # bf16 slab cache: 2 batch generations in flight (16 slabs x 8KB/part)
    cache = es.enter_context(tc.tile_pool(name="cache", bufs=2 * D))
    stage = es.enter_context(tc.tile_pool(name="stage", bufs=2))
    fxp = es.enter_context(tc.tile_pool(name="fxp", bufs=2))
    ppool = es.enter_context(tc.tile_pool(name="ppool", bufs=2))
    apool = es.enter_context(tc.tile_pool(name="apool", bufs=2))
    ystp = es.enter_context(tc.tile_pool(name="ystp", bufs=2))

    cc_in, cc_out = [], []
    for b in range(B):
        cc_in_b = dram.tile([G, D], F32, tag=f"ci{b}", name=f"cc_in_{b}")
        cc_out_b = dram.tile([G, D], F32, tag=f"co{b}", name=f"cc_out_{b}")
        cc_in.append(cc_in_b)
        cc_out.append(cc_out_b)

    psS = es.enter_context(tc.tile_pool(name="psumS", bufs=1, space="PSUM"))
    psT = es.enter_context(tc.tile_pool(name="psumT", bufs=2, space="PSUM"))
    psU = es.enter_context(tc.tile_pool(name="psumU", bufs=1, space="PSUM"))
    psF = es.enter_context(tc.tile_pool(name="psumF", bufs=1, space="PSUM"))
    psL = es.enter_context(tc.tile_pool(name="psumL", bufs=1, space="PSUM"))
    xtp = es.enter_context(tc.tile_pool(name="xtp", bufs=2))

    # one-time W_u transpose: wT[c, g] chunks (fp32 + bf16 copies)
    for k in range(NKC):
        tr = psF.tile([P, G], F32, tag="fix")
        nc.tensor.transpose(tr[:, :G], w_sb[:, k * P:(k + 1) * P], ident[:G, :G])
        nc.vector.tensor_copy(wT_sb[:, k, :], tr[:, :G])
        nc.scalar.copy(wT_bf[:, k, :], tr[:, :G])

    cache_tiles = {}            # (b, d) -> bf16 slab tile [P, nj, C]
    ystage = {}                 # (b, h) -> f32 tile [P, 2, C]

    def emit_slab(b, d, di, sums_ps):
        slab = stage.tile([P, nj, C], F32, tag="slab")
        nc.sync.dma_start(
            slab[:], x[d, b].rearrange("(p j) c -> p j c", j=nj))
        xbf = cache.tile([P, nj, C], BF16, tag="cslab")
        cache_tiles[(b, d)] = xbf
        # casts all on ACT: DVE is reserved for combines
        for j in range(nj):
            nc.scalar.copy(xbf[:, j, :], slab[:, j, :])
        # column sums over the slab's 512 rows into psum row d (PE)
        for h in range(2):
            for j in range(nj):
                nc.tensor.matmul(
                    sums_ps[:, h * 512:(h + 1) * 512],
                    ind_bf[:, d, :],
                    xbf[:, j, h * 512:(h + 1) * 512],
                    start=(di == 0 and j == 0),
                    stop=(di == D - 1 and j == nj - 1),
                )

    def emit_ublock(b, j):
        # uT[g, s-block] = sum_k (wT_k).T @ x7T_k on PE
        x7bf = cache_tiles[(b, D - 1)]
        u_ps = psU.tile([G, P], F32, tag="u")
        for k in range(NKC):
            xt_ps = psT.tile([P, P], BF16, tag="xt")
            nc.tensor.transpose(
                xt_ps[:], x7bf[:, j, k * P:(k + 1) * P], ident_bf[:])
            xt_sb = xtp.tile([P, P], BF16, tag="xt_sb")
            if k % 2 == 0:
                nc.scalar.copy(xt_sb[:], xt_ps[:])
            else:
                nc.vector.tensor_copy(xt_sb[:], xt_ps[:])
            nc.tensor.matmul(
                u_ps[:], wT_bf[:, k, :], xt_sb[:],
                start=(k == 0), stop=(k == NKC - 1))
        nc.scalar.copy(uT_sb[:, b, j, :], u_ps[:])

    def emit_fixup(b, sums_ps):
        # sums -> meanT chunks -> partial keysT; fixup copies on ACT so
        # the AR trigger is never gated by DVE combine backlog
        sums_sb = fxp.tile([D, C], F32, tag="sums_sb")
        nc.scalar.copy(sums_sb[:], sums_ps[:])
        mt_ps = psF.tile([P, NKC * D], F32, tag="fix")
        for k in range(NKC):
            nc.tensor.matmul(
                mt_ps[:, k * D:(k + 1) * D],
                sums_sb[:, k * P:(k + 1) * P], ident[:D, :D],
                is_transpose=True, start=(k == 0), stop=(k == NKC - 1))
        meanT_tmp = xtp.tile([P, NKC * D], F32, tag="mt")
        nc.scalar.copy(meanT_tmp[:], mt_ps[:])
        keys_ps = psF.tile([P, NKC * D], F32, tag="fix")
        for k in range(NKC):
            nc.tensor.matmul(
                keys_ps[:G, :D],
                wT_sb[:, k, :],
                meanT_tmp[:, k * D:(k + 1) * D],
                start=(k == 0), stop=(k == NKC - 1))
        nc.scalar.copy(sumk_sb[:, b, :], keys_ps[:G, :D])
        nc.gpsimd.dma_start(cc_in[b][:], sumk_sb[:, b, :])
        nc.gpsimd.collective_compute(
            "AllReduce", add, replica_groups=[list(range(N_CORES))],
            ins=[cc_in[b].opt()], outs=[cc_out[b].opt()],
        )

    def emit_prelude(b):
        # keysT fetch + logits + softmax for batch b
        nc.gpsimd.dma_start(keysT_sb[:, b, :], cc_out[b][:])
        nc.scalar.copy(keysT_bf[:, b, :], keysT_sb[:, b, :])
        for j in range(nj):
            lg_ps = psL.tile([P, D], F32, tag="lg")
            nc.tensor.matmul(lg_ps[:], uT_sb[:, b, j, :], keysT_bf[:, b, :])
            e_sb = xtp.tile([P, D], F32, tag="e")
            z_sb = xtp.tile([P, 1], F32, tag="z")
            rz_sb = xtp.tile([P, 1], F32, tag="rz")
            nc.scalar.activation(
                e_sb[:], lg_ps[:], mybir.ActivationFunctionType.Exp,
                accum_out=z_sb[:])
            nc.vector.reciprocal(rz_sb[:], z_sb[:])
            nc.scalar.mul(gates_sb[:, b, j, :], e_sb[:], rz_sb[:])

    def emit_chain(b, j):
        # weighted-depth combine for block j, all in bf16 so every op runs
        # in the DVE 2x fast mode (tensor_scalar / tensor_tensor; STT is
        # not fast-mode eligible): 8 products + 7 adds, final add emits
        # f32. bf16 accumulation costs ~0.5% relative — budget is 2e-2.
        # For the tail batch the products run on ACT (idle there), halving
        # the exposed tail combine.
        h, jj = j // 2, j % 2
        act_products = (b == B - 1)
        if jj == 0:
            yst_t = ystp.tile([P, 2, C], F32, tag="yst", name=f"yst_{b}_{h}")
            ystage[(b, h)] = yst_t
        yst = ystage[(b, h)]
        acc = apool.tile([P, C], BF16, tag="cacc")
        dorder = [D - 1] + list(range(D - 1))
        for di, d in enumerate(dorder):
            xbf = cache_tiles[(b, d)]
            g = gates_sb[:, b, j, d:d + 1]
            if di == 0:
                nc.vector.tensor_scalar_mul(acc[:], xbf[:, j, :], g)
                continue
            p = ppool.tile([P, C], BF16, tag="prod")
            if act_products:
                nc.scalar.mul(p[:], xbf[:, j, :], g)
            else:
                nc.vector.tensor_scalar_mul(p[:], xbf[:, j, :], g)
            if di < D - 1:
                nc.vector.tensor_add(acc[:], acc[:], p[:])
            else:
                nc.vector.tensor_add(yst[:, jj, :], acc[:], p[:])
        if j == 3:
            for d in range(D):
                del cache_tiles[(b, d)]

    def emit_ywrite(b, h):
        nc.gpsimd.dma_start(
            y[b].rearrange("(p j) c -> p j c", j=nj)[:, 2 * h:2 * h + 2, :],
            ystage.pop((b, h))[:])

    dorder = [D - 1] + list(range(D - 1))
    for b in range(B):
        sums_ps = psS.tile([D, C], F32, tag="sums")
        for si, d in enumerate(dorder):
            emit_slab(b, d, si, sums_ps)
            if 1 <= si <= nj:
                emit_ublock(b, si - 1)
            if b >= 1:
                if si == 5:
                    emit_prelude(b - 1)
                elif si == 6:
                    emit_chain(b - 1, 0)
                    emit_chain(b - 1, 1)
                    emit_ywrite(b - 1, 0)
                elif si == 7:
                    emit_chain(b - 1, 2)
        emit_fixup(b, sums_ps)
        if b >= 1:
            emit_chain(b - 1, 3)
            emit_ywrite(b - 1, 1)
    emit_prelude(B - 1)
    for j in range(nj):
        emit_chain(B - 1, j)
        if j % 2 == 1:
            emit_ywrite(B - 1, j // 2)

    es.close()


def build_nc(s_sh):
    nc = bacc.Bacc("TRN2", target_bir_lowering=False, debug=False,
                   num_devices=N_CORES)
    x_ap = nc.dram_tensor("x", [D, B, s_sh, C], F32, kind="ExternalInput").ap()
    w_ap = nc.dram_tensor("w", [G, C], F32, kind="ExternalInput").ap()
    y_ap = nc.dram_tensor("y", [B, s_sh, C], F32, kind="ExternalOutput").ap()
    with tile.TileContext(nc) as tc:
        build_body(tc, x_ap, w_ap, y_ap, s_sh)
    nc.compile()
    return nc


_NC_CACHE = {}


def _get_nc(s_sh):
    if s_sh not in _NC_CACHE:
        _NC_CACHE[s_sh] = build_nc(s_sh)
    return _NC_CACHE[s_sh]


def run(cached_states, W_u, trace=False, trace_cores=None):
    s_sh = S // N_CORES
    nc = _get_nc(s_sh)
    xs = np.asarray(cached_states, dtype=np.float32)
    ws = np.ascontiguousarray(np.asarray(W_u, dtype=np.float32))
    in_maps = []
    for i in range(N_CORES):
        sh = np.ascontiguousarray(xs[:, :, i * s_sh:(i + 1) * s_sh, :])
        in_maps.append({"x": sh, "w": ws})
    res = bass_utils.run_bass_kernel_spmd(
        nc, in_maps, core_ids=list(range(N_CORES)), trace=trace,
        trace_cores=trace_cores)
    out = np.empty((B, S, C), np.float32)
    for i in range(N_CORES):
        out[:, i * s_sh:(i + 1) * s_sh, :] = res.results[i]["y"]
    return out, res


def kernel(cached_states, W_u):
    out, _ = run(cached_states, W_u)
    return out
